# revision 38
# baseline (speedup 1.0000x reference)
"""Causal self-attention head (B=4, T=2048, D=768, H=64) on 8 TRN2 NeuronCores.

Sharding: 2 cores per batch element. Causal attention work grows with row
index, so core g in {0,1} of example b takes the interleaved 128-row q-tiles
(g=0: even tiles, g=1: odd tiles) -- perfectly balanced across the pair.

One uniform SPMD program for all 8 cores; per-core differences are pure data:
  - x^T is fed host-transposed (d on partitions) with a per-core *column
    block permutation* (g=0 uses block order [15,0,1,...,14]) so that the
    core's j-th q-tile always sits at permuted position 2j+1 and needs
    exactly the first 2j+2 key blocks -- uniform static loop bounds.
  - causal masks are per-core input data: a position-0 mask (zeros for g=0,
    whose position 0 holds the never-valid block 15; ones for g=1) plus a
    shared "last four blocks" mask M4 that handles both diagonal tiles of a
    256-wide q-chunk.

Compute (per core; every matmul contracts the partition dim):
  [kT; vT] = [Wk|Wv].T @ x^T  -- one M=128 matmul group per 512 cols,
  contracting d in 6 128-chunks (PSUM-accumulated); qT likewise but only for
  the core's own 1024 q-cols (strided rhs over odd position blocks).
  v_aug[s,65] blocks via PE-transpose of vT rows (col 64 preset to 1 so the
  softmax denominators fall out of the PV matmul as output row 64).
  Attention in 4 q-chunks of 256 cols (q-tile pairs), chunk c needing the
  first 4c+4 key blocks, interleaved with the projection t-groups so ACT/DVE
  work overlaps PE:
    S^T[s,t] = matmul(lhsT=kT block, rhs=qT pair)           [128 x 256]
    p = exp(S^T / 8) on ACT (logits bounded ~+-6: no max subtraction),
    causal-masked by multiply, then
    outT[65, 256] += matmul(lhsT=v_aug block, rhs=p block)   (PSUM accum)
  Epilogue per 128-col half: PE-transpose [65,128]->[128,65], reciprocal of
  col 64, scale, DMA natural-layout [128,64] rows to DRAM.
"""

import math
import numpy as np
import ml_dtypes

B, T, D, H = 4, 2048, 768, 64
P = 128
NT = T // P            # 16 key/query tile blocks
NCH = NT // 4          # 4 q-chunks per core (256 q-cols each)
DCH = D // P           # 6 d-chunks
TG = 512               # t-group width for projections
NTG = T // TG          # 4
VW = H + 1             # 65
WKV = DCH * P          # 768 cols of packed [Wk|Wv] chunks
WQK = DCH * P          # 768 cols of packed [Wq|Wk] chunks

_CACHE = {}


def _build_nc():
    import concourse.bacc as bacc
    import concourse.tile as tile
    import concourse.mybir as mybir

    f32 = mybir.dt.float32
    bf16 = mybir.dt.bfloat16

    nc = bacc.Bacc("TRN2", debug=False, num_devices=8)

    # host-prepacked layouts (see _make_in_maps)
    xt = nc.dram_tensor("xt", [D, T], bf16, kind="ExternalInput")
    w3 = nc.dram_tensor("w3", [P, WKV + WQK], bf16, kind="ExternalInput")
    bias2 = nc.dram_tensor("bias2", [P, 2], f32, kind="ExternalInput")
    msk = nc.dram_tensor("msk", [P, 10 * P], bf16, kind="ExternalInput")
    idn = nc.dram_tensor("idn", [P, H], bf16, kind="ExternalInput")
    idnf = nc.dram_tensor("idnf", [VW, VW], f32, kind="ExternalInput")
    out = nc.dram_tensor("out", [8 * P, H], f32, kind="ExternalOutput")

    with tile.TileContext(nc) as tc:
        with (
            tc.tile_pool(name="const", bufs=1) as constp,
            tc.tile_pool(name="ptp", bufs=4) as ptp,
            tc.tile_pool(name="smp", bufs=3) as smp,
            tc.tile_pool(name="projp", bufs=2, space="PSUM") as projp,
            tc.tile_pool(name="tpp", bufs=1, space="PSUM") as tpp,
            tc.tile_pool(name="stp", bufs=2, space="PSUM") as stp,
            tc.tile_pool(name="otp", bufs=2, space="PSUM") as otp,
        ):
            # ---------------- weights first, then tg-major xt pieces ------------
            # DMA issue costs ~640ns each on the issuing engine, so ordering
            # decides when the first projection can start.
            w_sb = constp.tile([P, WKV + WQK], bf16, tag="w3")
            nc.sync.dma_start(w_sb[:, 0:WKV], w3[:, 0:WKV])
            nc.scalar.dma_start(w_sb[:, WKV:], w3[:, WKV:])

            xt_sb = [
                constp.tile([P, T], bf16, tag=f"xt{c}", name=f"xt_sb{c}")
                for c in range(DCH)
            ]
            dma_engines = [nc.scalar, nc.sync]
            di = 0
            for tg in range(NTG):
                for c in range(DCH):
                    eng = dma_engines[di % len(dma_engines)]
                    di += 1
                    eng.dma_start(
                        xt_sb[c][:, tg * TG:(tg + 1) * TG],
                        xt[c * P:(c + 1) * P, tg * TG:(tg + 1) * TG],
                    )

            # remaining constants on gpsimd (SWDGE) to keep HWDGE engines free
            b_sb = constp.tile([P, 2], f32, tag="b2")
            nc.gpsimd.dma_start(b_sb[:, :], bias2[:, :])
            idn_sb = constp.tile([P, H], bf16, tag="idn")
            nc.gpsimd.dma_start(idn_sb[:, :], idn[:, :])
            msk_sb = constp.tile([P, 10 * P], bf16, tag="msk")
            nc.gpsimd.dma_start(msk_sb[:, :], msk[:, :])
            idnf_sb = constp.tile([VW, VW], f32, tag="idnf")
            nc.gpsimd.dma_start(idnf_sb[:, :], idnf[:, :])

            kvt_sb = constp.tile([P, T], bf16, tag="kvt")  # rows 0:64 kT, 64:128 vT
            # rows 0:64 qT (odd-position q-cols, slot-major); rows 64:128 kT of
            # the same odd position blocks (for row-group-packed S^T matmuls)
            qk_sb = constp.tile([H, 8 * P], bf16, tag="qk")
            v_sb = constp.tile([P, NT * VW], bf16, tag="v")
            # ones column (col 64 of every v block) via one strided memset
            v_ones = v_sb[:, :].rearrange("p (s e) -> p s e", e=VW)[:, :, H:VW]
            nc.vector.memset(v_ones, 1.0)

            # odd-position view of xt chunks for the q projection
            xt_v = [
                xt_sb[c][:, :].rearrange("p (b two k) -> p b two k", two=2, k=P)
                for c in range(DCH)
            ]

            for tg in range(NTG):
                # ---- [kT; vT] projection for this 512-col t-group ----
                ps = projp.tile([P, TG], f32, tag="proj")
                for c in range(DCH):
                    nc.tensor.matmul(
                        ps[:, :],
                        lhsT=w_sb[:, c * P:(c + 1) * P],
                        rhs=xt_sb[c][:, tg * TG:(tg + 1) * TG],
                        start=(c == 0),
                        stop=(c == DCH - 1),
                    )
                nc.vector.tensor_scalar_add(
                    kvt_sb[:, tg * TG:(tg + 1) * TG], ps[:, :], b_sb[:, 0:1]
                )
                # ---- qT for this t-group's two odd position blocks ----
                qs_ps = projp.tile([H, 2 * P], f32, tag="qproj", bufs=1)
                for c in range(DCH):
                    nc.tensor.matmul(
                        qs_ps[:, :],
                        lhsT=w_sb[:, WKV + c * P:WKV + c * P + H],
                        rhs=xt_v[c][:, 2 * tg:2 * tg + 2, 1:2, :],
                        start=(c == 0),
                        stop=(c == DCH - 1),
                    )
                nc.vector.tensor_scalar_add(
                    qk_sb[0:H, tg * 2 * P:(tg + 1) * 2 * P], qs_ps[:, :],
                    b_sb[0:H, 1:2],
                )
                # ---- v_aug blocks for this t-group ----
                for s in range(4 * tg, 4 * tg + 4):
                    vp = tpp.tile([P, H], bf16, tag="tp")
                    nc.tensor.transpose(
                        vp[:, :],
                        kvt_sb[H:P, s * P:(s + 1) * P],
                        idn_sb[H:P, :],
                    )
                    nc.vector.tensor_copy(v_sb[:, s * VW:s * VW + H], vp[:, :])

                # ---- attention chunk c = tg (needs blocks < 4c+4 ✓ just made) --
                c = tg
                nb = 4 * c + 4
                ot = otp.tile([VW, 2 * P], f32, tag="ot")
                qs_lo = qk_sb[0:H, c * 2 * P:(c + 1) * 2 * P]
                for grp in range(nb // 2):
                    st = stp.tile([P, 4 * P], f32, tag="st")
                    nc.tensor.matmul(
                        st[:, 0:2 * P],
                        lhsT=kvt_sb[0:H, 2 * grp * P:(2 * grp + 1) * P],
                        rhs=qs_lo,
                        start=True,
                        stop=True,
                    )
                    nc.tensor.matmul(
                        st[:, 2 * P:4 * P],
                        lhsT=kvt_sb[0:H, (2 * grp + 1) * P:(2 * grp + 2) * P],
                        rhs=qs_lo,
                        start=True,
                        stop=True,
                    )
                    pt = ptp.tile([P, 4 * P], bf16, tag="pt")
                    nc.scalar.activation(
                        pt[:, :], st[:, :],
                        mybir.ActivationFunctionType.Exp,
                        scale=1.0 / math.sqrt(H),
                    )
                    for k in (0, 1):
                        s = 2 * grp + k
                        sl = pt[:, k * 2 * P:(k + 1) * 2 * P]
                        if s == 0:
                            nc.vector.tensor_mul(sl, sl, msk_sb[:, 0:2 * P])
                        if s >= nb - 4:
                            d = s - (nb - 4)
                            nc.vector.tensor_mul(
                                sl, sl,
                                msk_sb[:, (1 + d) * 2 * P:(2 + d) * 2 * P],
                            )
                    for k in (0, 1):
                        s = 2 * grp + k
                        nc.tensor.matmul(
                            ot[:, :],
                            lhsT=v_sb[:, s * VW:(s + 1) * VW],
                            rhs=pt[:, k * 2 * P:(k + 1) * 2 * P],
                            start=(s == 0),
                            stop=(s == nb - 1),
                        )
                # epilogue: two 128-col halves
                osb = smp.tile([VW, 2 * P], f32, tag="osb")
                nc.vector.tensor_copy(osb[:, :], ot[:, :])
                for half in range(2):
                    ep = tpp.tile([P, VW], f32, tag="tp")
                    nc.tensor.transpose(
                        ep[:, :], osb[:, half * P:(half + 1) * P], idnf_sb[:, :]
                    )
                    rc = smp.tile([P, 1], f32, tag="rc")
                    nc.vector.reciprocal(rc[:, :], ep[:, H:VW])
                    ob = smp.tile([P, H], f32, tag="ob")
                    nc.vector.tensor_scalar_mul(ob[:, :], ep[:, 0:H], rc[:, :])
                    r = 2 * c + half
                    nc.sync.dma_start(out[r * P:(r + 1) * P, :], ob[:, :])

    nc.compile()
    return nc


def _perm_blocks(g):
    if g == 1:
        return list(range(NT))
    return [NT - 1] + list(range(NT - 1))


def _make_masks():
    """M0 variants + shared M4 [128, 4*256] (d-th block, two 128 t-halves)."""
    one = np.ones((P, P), np.float32)
    zero = np.zeros((P, P), np.float32)
    tri = np.triu(np.ones((P, P), np.float32))  # [s,t]: 1 if s <= t
    m4 = np.concatenate(
        [
            np.concatenate([one, one], axis=1),
            np.concatenate([tri, one], axis=1),
            np.concatenate([zero, one], axis=1),
            np.concatenate([zero, tri], axis=1),
        ],
        axis=1,
    )  # [128, 1024]
    return zero, one, m4


def _make_in_maps(x, Wq, bq_, Wk, bk_, Wv, bv_):
    bf16 = ml_dtypes.bfloat16
    zero, one, m4 = _make_masks()

    # w3 = [ packed [Wk|Wv] chunks [128, 768] | packed [Wq|Wk] chunks ]
    w3 = np.empty((P, WKV + WQK), np.float32)
    for c in range(DCH):
        w3[:, c * P:c * P + H] = Wk[c * P:(c + 1) * P, :]
        w3[:, c * P + H:(c + 1) * P] = Wv[c * P:(c + 1) * P, :]
        w3[:, WKV + c * P:WKV + c * P + H] = Wq[c * P:(c + 1) * P, :]
        w3[:, WKV + c * P + H:WKV + (c + 1) * P] = Wk[c * P:(c + 1) * P, :]
    # bias2 col0: rows 0:64 bk, rows 64:128 bv (matches kv psum layout);
    # col1: rows 0:64 bq, rows 64:128 bk (matches qk psum layout)
    bias2 = np.zeros((P, 2), np.float32)
    bias2[0:H, 0] = bk_
    bias2[H:P, 0] = bv_
    bias2[0:H, 1] = bq_
    bias2[H:P, 1] = bk_
    # identity: rows 64:128 used for the vT transpose (lhsT lives at base 64)
    idn = np.zeros((P, H), np.float32)
    idn[0:H] = np.eye(H)
    idn[H:P] = np.eye(H)
    identf = np.eye(VW, dtype=np.float32)

    common = {
        "w3": np.ascontiguousarray(w3.astype(bf16)),
        "bias2": np.ascontiguousarray(bias2),
        "idn": np.ascontiguousarray(idn.astype(bf16)),
        "idnf": np.ascontiguousarray(identf),
    }
    in_maps = []
    for core in range(2 * B):
        b, g = core // 2, core % 2
        perm = _perm_blocks(g)
        cols = np.concatenate([np.arange(blk * P, (blk + 1) * P) for blk in perm])
        xt_np = np.ascontiguousarray(x[b].T[:, cols].astype(bf16))
        m0 = one if g == 1 else zero
        msk_np = np.concatenate([m0, m0, m4], axis=1)  # [128, 1280]
        in_maps.append(
            dict(common, xt=xt_np, msk=np.ascontiguousarray(msk_np.astype(bf16)))
        )
    return in_maps


def _gather(results, x_dtype):
    out = np.empty((B, T, H), np.float32)
    for core in range(2 * B):
        b, g = core // 2, core % 2
        oc = results[core]["out"]
        for j in range(8):
            a = 2 * j + g
            out[b, a * P:(a + 1) * P, :] = oc[j * P:(j + 1) * P, :]
    return out.astype(x_dtype, copy=False)


def run(inputs, trace=False):
    """Build (cached), run on 8 cores, return (full_output, BassKernelResults)."""
    from concourse.bass_utils import run_bass_kernel_spmd

    if "nc" not in _CACHE:
        _CACHE["nc"] = _build_nc()
    nc = _CACHE["nc"]
    in_maps = _make_in_maps(
        np.asarray(inputs["x"]),
        np.asarray(inputs["Wq"]), np.asarray(inputs["bq"]),
        np.asarray(inputs["Wk"]), np.asarray(inputs["bk"]),
        np.asarray(inputs["Wv"]), np.asarray(inputs["bv"]),
    )
    kwargs = {}
    if trace:
        kwargs = dict(trace=True, stitch_traces=True, trace_cores=list(range(2 * B)))
    res = run_bass_kernel_spmd(nc, in_maps, core_ids=list(range(2 * B)), **kwargs)
    out = _gather(res.results, np.asarray(inputs["x"]).dtype)
    return out, res


def kernel(**inputs) -> np.ndarray:
    out, _ = run(inputs, trace=False)
    return out


# revision 76
# speedup vs baseline: 1.3565x; 1.3565x over previous
"""Causal self-attention head (B=4, T=2048, D=768, H=64) on 8 TRN2 NeuronCores.

Sharding: 2 cores per batch element. Causal attention work grows with row
index, so core g in {0,1} of example b takes the interleaved 128-row q-tiles
(g=0: even tiles, g=1: odd tiles) -- perfectly balanced across the pair.

One uniform SPMD program for all 8 cores; per-core differences are pure data:
  - x^T is fed host-transposed (d on partitions) with a per-core *column
    block permutation* (g=0 uses block order [15,0,1,...,14]) so that the
    core's j-th q-tile always sits at permuted position 2j+1 and needs
    exactly the first 2j+2 key blocks -- uniform static loop bounds.
  - causal masks are per-core input data: a position-0 mask (zeros for g=0,
    whose position 0 holds the never-valid block 15; ones for g=1) plus a
    shared "last four blocks" mask M4 that handles both diagonal tiles of a
    256-wide q-chunk.

Compute (per core; every matmul contracts the partition dim):
  [kT; vT] = [Wk|Wv].T @ x^T  -- one M=128 matmul group per 512 cols,
  contracting d in 6 128-chunks (PSUM-accumulated); qT likewise but only for
  the core's own 1024 q-cols (strided rhs over odd position blocks).
  v_aug[s,65] blocks via PE-transpose of vT rows (col 64 preset to 1 so the
  softmax denominators fall out of the PV matmul as output row 64).
  Attention in 4 q-chunks of 256 cols (q-tile pairs), chunk c needing the
  first 4c+4 key blocks, interleaved with the projection t-groups so ACT/DVE
  work overlaps PE:
    S^T[s,t] = matmul(lhsT=kT block, rhs=qT pair)           [128 x 256]
    p = exp(S^T / 8) on ACT (logits bounded ~+-6: no max subtraction),
    causal-masked by multiply, then
    outT[65, 256] += matmul(lhsT=v_aug block, rhs=p block)   (PSUM accum)
  Epilogue per 128-col half: PE-transpose [65,128]->[128,65], reciprocal of
  col 64, scale, DMA natural-layout [128,64] rows to DRAM.
"""

import math
import numpy as np
import ml_dtypes

B, T, D, H = 4, 2048, 768, 64
P = 128
NT = T // P            # 16 key/query tile blocks
NCH = NT // 4          # 4 q-chunks per core (256 q-cols each)
DCH = D // P           # 6 d-chunks
TG = 512               # t-group width for projections
NTG = T // TG          # 4
VW = H + 1             # 65
WKV = DCH * P          # 768 cols of packed [Wk|Wv] chunks
WQK = DCH * P          # 768 cols of packed [Wq|Wk] chunks

_CACHE = {}


def _build_nc():
    import concourse.bacc as bacc
    import concourse.tile as tile
    import concourse.mybir as mybir

    f32 = mybir.dt.float32
    bf16 = mybir.dt.bfloat16

    nc = bacc.Bacc("TRN2", debug=False, num_devices=8, enable_partition_id=False)

    # host-prepacked layouts (see _make_in_maps)
    xt = nc.dram_tensor("xt", [D, T], bf16, kind="ExternalInput")
    w3 = nc.dram_tensor("w3", [P, WKV + WQK], bf16, kind="ExternalInput")
    bias2 = nc.dram_tensor("bias2", [P, 3], f32, kind="ExternalInput")
    msk = nc.dram_tensor("msk", [P, 8 * P], bf16, kind="ExternalInput")
    idn = nc.dram_tensor("idn", [P, H], bf16, kind="ExternalInput")
    idnf = nc.dram_tensor("idnf", [VW, VW], f32, kind="ExternalInput")
    out = nc.dram_tensor("out", [8 * P, H], f32, kind="ExternalOutput")

    with tile.TileContext(nc) as tc:
        with (
            tc.tile_pool(name="const", bufs=1) as constp,
            tc.tile_pool(name="ptp", bufs=4) as ptp,
            tc.tile_pool(name="smp", bufs=3) as smp,
            tc.tile_pool(name="projp", bufs=1, space="PSUM") as projp,
            tc.tile_pool(name="tpp", bufs=2, space="PSUM") as tpp,
            tc.tile_pool(name="stp", bufs=3, space="PSUM") as stp,
            tc.tile_pool(name="otp", bufs=1, space="PSUM") as otp,
        ):
            # ---------------- weights first, then tg-major xt pieces ------------
            # DMA issue costs ~640ns each on the issuing engine, so ordering
            # decides when the first projection can start.
            w_sb = constp.tile([P, WKV + WQK], bf16, tag="w3")
            nc.sync.dma_start(w_sb[:, 0:WKV // 2], w3[:, 0:WKV // 2])
            nc.scalar.dma_start(w_sb[:, WKV // 2:WKV], w3[:, WKV // 2:WKV])

            xt_sb = [
                constp.tile([P, T], bf16, tag=f"xt{c}", name=f"xt_sb{c}")
                for c in range(DCH)
            ]
            dma_engines = [nc.scalar, nc.sync]
            di = 0
            for tg in range(NTG):
                # tg0 in 256-col halves: 12 parallel queues so the first
                # projection group's last-arriving chunk lands sooner
                pieces = (0, 256, 512) if tg == 0 else (0, 512)
                for c in range(DCH):
                    eng = dma_engines[di % len(dma_engines)]
                    di += 1
                    for lo, hi in zip(pieces, pieces[1:]):
                        eng.dma_start(
                            xt_sb[c][:, tg * TG + lo:tg * TG + hi],
                            xt[c * P:(c + 1) * P, tg * TG + lo:tg * TG + hi],
                        )
                if tg == 0:
                    # qk weights after the first t-group's x pieces: needed
                    # only once the kv projection of tg0 is underway
                    nc.sync.dma_start(
                        w_sb[:, WKV:WKV + WQK // 2], w3[:, WKV:WKV + WQK // 2]
                    )
                    nc.scalar.dma_start(
                        w_sb[:, WKV + WQK // 2:], w3[:, WKV + WQK // 2:]
                    )

            # remaining constants on gpsimd (SWDGE) to keep HWDGE engines free
            b_sb = constp.tile([P, 3], f32, tag="b2")
            nc.gpsimd.dma_start(b_sb[:, :], bias2[:, :])
            idn_sb = constp.tile([P, H], bf16, tag="idn")
            nc.gpsimd.dma_start(idn_sb[:, :], idn[:, :])
            msk_sb = constp.tile([P, 8 * P], bf16, tag="msk")
            nc.gpsimd.dma_start(msk_sb[:, :], msk[:, :])
            idnf_sb = constp.tile([VW, VW], f32, tag="idnf")
            nc.gpsimd.dma_start(idnf_sb[:, :], idnf[:, :])

            # PE warm-up: the tensor engine is DMA-starved for the first ~10us
            # and would then pay the HAM half-clock ramp on real work. Stream
            # junk matmuls (unwritten scratch tile, discarded PSUM) to hold the
            # activity monitor at full clock until the first projection.
            scr_sb = constp.tile([P, TG], bf16, tag="scr")
            nc.vector.memset(scr_sb[:, :], 1.0)
            for wi in range(9):
                wps = projp.tile([P, TG], f32, tag="qproj", bufs=1, name=f"wps{wi}")
                nc.tensor.matmul(
                    wps[:, :], lhsT=scr_sb[:, 0:P], rhs=scr_sb[:, :],
                    start=True, stop=True,
                )

            kvt_sb = constp.tile([P, T], bf16, tag="kvt")  # rows 0:64 kT, 64:128 vT
            # rows 0:64 qT (odd-position q-cols, slot-major); rows 64:128 kT of
            # the same odd position blocks (for row-group-packed S^T matmuls)
            qk_sb = constp.tile([H, 8 * P], bf16, tag="qk")
            v_sb = constp.tile([P, NT * VW], bf16, tag="v")
            # ones column (col 64 of every v block -> sums on PSUM partition 64)
            v_ones = v_sb[:, :].rearrange("p (s e) -> p s e", e=VW)[:, :, H:VW]
            nc.vector.memset(v_ones, 1.0)

            # odd-position view of xt chunks for the q projection
            xt_v = [
                xt_sb[c][:, :].rearrange("p (b two k) -> p b two k", two=2, k=P)
                for c in range(DCH)
            ]

            for tg in range(NTG):
                # ---- [kT; vT] projection for this 512-col t-group ----
                ps = projp.tile([P, TG], f32, tag="proj")
                for c in range(DCH):
                    nc.tensor.matmul(
                        ps[:, :],
                        lhsT=w_sb[:, c * P:(c + 1) * P],
                        rhs=xt_sb[c][:, tg * TG:(tg + 1) * TG],
                        start=(c == 0),
                        stop=(c == DCH - 1),
                    )
                nc.vector.tensor_scalar_add(
                    kvt_sb[:, tg * TG:(tg + 1) * TG], ps[:, :], b_sb[:, 0:1]
                )
                # ---- qT for this t-group's two odd position blocks ----
                qs_ps = projp.tile([H, 2 * P], f32, tag="qproj", bufs=1)
                for c in range(DCH):
                    nc.tensor.matmul(
                        qs_ps[:, :],
                        lhsT=w_sb[:, WKV + c * P:WKV + c * P + H],
                        rhs=xt_v[c][:, 2 * tg:2 * tg + 2, 1:2, :],
                        start=(c == 0),
                        stop=(c == DCH - 1),
                    )
                nc.vector.tensor_scalar_add(
                    qk_sb[0:H, tg * 2 * P:(tg + 1) * 2 * P], qs_ps[:, :],
                    b_sb[0:H, 1:2],
                )
                # ---- v_aug blocks for this t-group ----
                for s in range(4 * tg, 4 * tg + 4):
                    vp = tpp.tile([P, H], bf16, tag="tp")
                    nc.tensor.transpose(
                        vp[:, :],
                        kvt_sb[H:P, s * P:(s + 1) * P],
                        idn_sb[H:P, :],
                    )
                    nc.vector.tensor_copy(v_sb[:, s * VW:s * VW + H], vp[:, :])

                # ---- attention chunk c = tg (needs blocks < 4c+4 ✓ just made) --
                c = tg
                nb = 4 * c + 4
                ot = otp.tile([VW, 2 * P], f32, tag="ot")
                qs_lo = qk_sb[0:H, c * 2 * P:(c + 1) * 2 * P]
                for grp in range(nb // 2):
                    st = stp.tile([P, 4 * P], f32, tag="st")
                    nc.tensor.matmul(
                        st[:, 0:2 * P],
                        lhsT=kvt_sb[0:H, 2 * grp * P:(2 * grp + 1) * P],
                        rhs=qs_lo,
                        start=True,
                        stop=True,
                    )
                    nc.tensor.matmul(
                        st[:, 2 * P:4 * P],
                        lhsT=kvt_sb[0:H, (2 * grp + 1) * P:(2 * grp + 2) * P],
                        rhs=qs_lo,
                        start=True,
                        stop=True,
                    )
                    pt = ptp.tile([P, 4 * P], bf16, tag="pt")
                    nc.scalar.activation(
                        pt[:, :], st[:, :],
                        mybir.ActivationFunctionType.Exp,
                        scale=1.0 / math.sqrt(H),
                    )
                    # causal masks: position-0 validity is a per-core 0/1
                    # scalar (bias2 col 2); M4 handles the last 4 blocks
                    if grp == 0:
                        nc.vector.tensor_scalar_mul(
                            pt[:, 0:2 * P], pt[:, 0:2 * P], b_sb[:, 2:3]
                        )
                    if grp >= nb // 2 - 2:
                        d = 2 * (grp - (nb // 2 - 2))  # 0 or 2
                        nc.vector.tensor_mul(
                            pt[:, :], pt[:, :],
                            msk_sb[:, d * 2 * P:(2 + d) * 2 * P],
                        )
                    for k in (0, 1):
                        s = 2 * grp + k
                        nc.tensor.matmul(
                            ot[:, :],
                            lhsT=v_sb[:, s * VW:(s + 1) * VW],
                            rhs=pt[:, k * 2 * P:(k + 1) * 2 * P],
                            start=(s == 0),
                            stop=(s == nb - 1),
                        )
                # epilogue: two 128-col halves, transpose to natural layout,
                # one merged output DMA per chunk
                osb = smp.tile([VW, 2 * P], f32, tag="osb")
                nc.vector.tensor_copy(osb[:, :], ot[:, :])
                ob = smp.tile([P, 2 * H], f32, tag="ob")
                for half in range(2):
                    ep = tpp.tile([P, VW], f32, tag="tp")
                    nc.tensor.transpose(
                        ep[:, :], osb[:, half * P:(half + 1) * P], idnf_sb[:, :]
                    )
                    rc = smp.tile([P, 1], f32, tag="rc")
                    nc.vector.reciprocal(rc[:, :], ep[:, H:VW])
                    nc.vector.tensor_scalar_mul(
                        ob[:, half * H:(half + 1) * H], ep[:, 0:H], rc[:, :]
                    )
                out_v = out[2 * c * P:(2 * c + 2) * P, :].rearrange(
                    "(h p) e -> p h e", p=P
                )
                nc.sync.dma_start(out_v, ob[:, :].rearrange("p (h e) -> p h e", e=H))

    nc.compile()
    return nc


def _perm_blocks(g):
    if g == 1:
        return list(range(NT))
    return [NT - 1] + list(range(NT - 1))


def _make_masks():
    """M0 variants + shared M4 [128, 4*256] (d-th block, two 128 t-halves)."""
    one = np.ones((P, P), np.float32)
    zero = np.zeros((P, P), np.float32)
    tri = np.triu(np.ones((P, P), np.float32))  # [s,t]: 1 if s <= t
    m4 = np.concatenate(
        [
            np.concatenate([one, one], axis=1),
            np.concatenate([tri, one], axis=1),
            np.concatenate([zero, one], axis=1),
            np.concatenate([zero, tri], axis=1),
        ],
        axis=1,
    )  # [128, 1024]
    return zero, one, m4


def _make_in_maps(x, Wq, bq_, Wk, bk_, Wv, bv_):
    bf16 = ml_dtypes.bfloat16
    zero, one, m4 = _make_masks()

    # w3 = [ packed [Wk|Wv] chunks [128, 768] | packed [Wq|Wk] chunks ]
    w3 = np.empty((P, WKV + WQK), np.float32)
    for c in range(DCH):
        w3[:, c * P:c * P + H] = Wk[c * P:(c + 1) * P, :]
        w3[:, c * P + H:(c + 1) * P] = Wv[c * P:(c + 1) * P, :]
        w3[:, WKV + c * P:WKV + c * P + H] = Wq[c * P:(c + 1) * P, :]
        w3[:, WKV + c * P + H:WKV + (c + 1) * P] = Wk[c * P:(c + 1) * P, :]
    # bias2 col0: rows 0:64 bk, rows 64:128 bv (matches kv psum layout);
    # col1: rows 0:64 bq, rows 64:128 bk (matches qk psum layout);
    # col2: per-core position-0 validity scalar (set per core below)
    bias2 = np.zeros((P, 3), np.float32)
    bias2[0:H, 0] = bk_
    bias2[H:P, 0] = bv_
    bias2[0:H, 1] = bq_
    bias2[H:P, 1] = bk_
    # identity: rows 64:128 used for the vT transpose (lhsT lives at base 64)
    idn = np.zeros((P, H), np.float32)
    idn[0:H] = np.eye(H)
    idn[H:P] = np.eye(H)
    identf = np.eye(VW, dtype=np.float32)

    common = {
        "w3": np.ascontiguousarray(w3.astype(bf16)),
        "idn": np.ascontiguousarray(idn.astype(bf16)),
        "idnf": np.ascontiguousarray(identf),
        "msk": np.ascontiguousarray(m4.astype(bf16)),  # [128, 1024]
    }
    in_maps = []
    for core in range(2 * B):
        b, g = core // 2, core % 2
        perm = _perm_blocks(g)
        cols = np.concatenate([np.arange(blk * P, (blk + 1) * P) for blk in perm])
        xt_np = np.ascontiguousarray(x[b].T[:, cols].astype(bf16))
        b2 = bias2.copy()
        b2[:, 2] = float(g)  # position-0 block valid only for g=1
        in_maps.append(dict(common, xt=xt_np, bias2=np.ascontiguousarray(b2)))
    return in_maps


def _gather(results, x_dtype):
    out = np.empty((B, T, H), np.float32)
    for core in range(2 * B):
        b, g = core // 2, core % 2
        oc = results[core]["out"]  # [1024, 64]
        for j in range(8):
            a = 2 * j + g
            out[b, a * P:(a + 1) * P, :] = oc[j * P:(j + 1) * P, :]
    return out.astype(x_dtype, copy=False)


def run(inputs, trace=False):
    """Build (cached), run on 8 cores, return (full_output, BassKernelResults)."""
    from concourse.bass_utils import run_bass_kernel_spmd

    if "nc" not in _CACHE:
        _CACHE["nc"] = _build_nc()
    nc = _CACHE["nc"]
    in_maps = _make_in_maps(
        np.asarray(inputs["x"]),
        np.asarray(inputs["Wq"]), np.asarray(inputs["bq"]),
        np.asarray(inputs["Wk"]), np.asarray(inputs["bk"]),
        np.asarray(inputs["Wv"]), np.asarray(inputs["bv"]),
    )
    kwargs = {}
    if trace:
        kwargs = dict(trace=True, stitch_traces=True, trace_cores=list(range(2 * B)))
    res = run_bass_kernel_spmd(nc, in_maps, core_ids=list(range(2 * B)), **kwargs)
    out = _gather(res.results, np.asarray(inputs["x"]).dtype)
    return out, res


def kernel(**inputs) -> np.ndarray:
    out, _ = run(inputs, trace=False)
    return out
